# revision 4
# baseline (speedup 1.0000x reference)
"""Trainium2 Bass kernel for nn_MemoryUnit (cosine-sim memory read with sparse
softmax shrinkage), data-parallel over 8 NeuronCores.

Per core (batch shard of 1024 rows), single fused pipeline:

  prologue : load x tiles, cast fp16, DMA-xbar transpose -> xT resident
             [f,b]; row norms -> invz (folded into logit evict, not into x).
  fused A+B1 (m-chunk outer, 512 cols = 4 mem row-tiles per chunk):
             stream mem chunk, row norms (sqrt + Newton), normalized fp16,
             transpose -> mhatT chunk (rolling, bufs=2).  As soon as chunk c
             is up: 8 bt x 16 k matmuls accumulate logits[bt, chunk] in 8
             psum banks.  Evict: DVE copy psum*invz -> l16 (fp16 logit
             store; logits ~0 near the mask threshold so fp16 is exact
             enough), ScalarE exp(psum*invz) dumped with accum_out ->
             per-chunk row sums sacc.
  threshold: per bt (pipelined under the last chunk's matmuls):
             T = thr*S, lnT by 3-term Taylor (|T-1|<~0.01), e16=exp(l16),
             v16 = (l16 > lnT) * e16 with accum -> vsum, invV = 1/vsum.
             v transposed -> vT [m,b] resident (sync queue, overlaps B1 tail).
  B2       : out[b,f] = sum_m vT[m,b] * mem[m,f]; mem re-streamed from HBM
             with casting gpsimd DMA (fp32->fp16, no bounce buffer), fc-outer
             (4 waves of 512 f-cols) x 32 m k-tiles x 8 bt; evict scaled by
             invV (softmax S cancels algebraically).

Threshold identity: relu(w-t)*w/(|w-t|+1e-12) == w * 1{w>t} to ~1e-7 rel,
w = e/S, so mask is e > t*S <=> logit > ln(t*S); final L1 norm reduces to
division by sum(v).
"""
import sys

sys.path.insert(0, "/opt/trn_rl_repo")

import numpy as np

N_CORES = 8
B_FULL = 8192
B = B_FULL // N_CORES    # 1024 batch rows per core
M = 4000                 # memory rows
MP = 4096                # padded memory rows (transpose granularity)
F = 2048                 # features
P = 128

_CACHE = {}


def build_nc(B=B, M=M, MP=MP, F=F):
    import concourse.bacc as bacc
    import concourse.mybir as mybir
    import concourse.tile as tile

    fp32 = mybir.dt.float32
    fp16 = mybir.dt.float16
    AF = mybir.ActivationFunctionType
    OP = mybir.AluOpType

    KT = F // P              # 16 k-tiles (contraction over features)
    BT = B // P              # 8 batch tiles per core
    MT = MP // P             # 32 padded memory row-tiles
    NCH = MP // 512          # 8 m-chunks of 512 cols for B1
    FC = F // 512            # 4 f-chunks of 512 cols for B2
    thr = 1.0 / M

    nc = bacc.Bacc("TRN2", target_bir_lowering=False, debug=True)
    with tile.TileContext(nc) as tc:
        with tc.tile_pool(name="dram", bufs=1, space="DRAM") as dram:
            xs = dram.tile([B, F], fp32, kind="ExternalInput", uniquify=False, name="xs")
            memory = dram.tile([M, F], fp32, kind="ExternalInput", uniquify=False, name="memory")
            out = dram.tile([B, F], fp32, kind="ExternalOutput", uniquify=False, name="out")

            with tc.tile_pool(name="ps", bufs=8, space="PSUM") as ps, \
                 tc.tile_pool(name="stats", bufs=1) as stats, \
                 tc.tile_pool(name="sml", bufs=4) as sml, \
                 tc.tile_pool(name="dmp", bufs=2) as dmp, \
                 tc.tile_pool(name="l16p", bufs=1) as l16p, \
                 tc.tile_pool(name="e16p", bufs=2) as e16p, \
                 tc.tile_pool(name="v16p", bufs=3) as v16p:

                eps = stats.tile([P, 1], fp32)
                nc.gpsimd.memset(eps[:], 1e-30)
                invz = stats.tile([P, BT], fp32)
                invV = stats.tile([P, BT], fp32)
                sacc = stats.tile([P, BT * NCH], fp32)
                l16 = l16p.tile([P, BT, M], fp16)   # fp16 logit store

                vts = [None] * BT    # v16 tiles pending transpose
                ivs = [None] * BT

                def thr_part(bt):
                    # T = thr * S (S = per-row exp sum over all m)
                    ds = sml.tile([P, NCH], fp32, tag="ds", bufs=2)
                    Tt = sml.tile([P, 1], fp32, tag="Tt", bufs=2)
                    nc.vector.tensor_scalar(
                        out=ds[:], in0=sacc[:, bt * NCH:(bt + 1) * NCH],
                        scalar1=thr, scalar2=0.0, op0=OP.mult, op1=OP.add,
                        accum_out=Tt[:])
                    # lnT = u - u^2/2 + u^3/3, u = T-1  (|u| ~ 1e-2)
                    u = sml.tile([P, 1], fp32, tag="u", bufs=2)
                    nc.vector.tensor_scalar_add(u[:], Tt[:], -1.0)
                    h = sml.tile([P, 1], fp32, tag="h", bufs=2)
                    nc.vector.tensor_scalar(
                        out=h[:], in0=u[:], scalar1=1.0 / 3.0, scalar2=0.5,
                        op0=OP.mult, op1=OP.subtract)
                    h2 = sml.tile([P, 1], fp32, tag="h2", bufs=2)
                    nc.vector.scalar_tensor_tensor(
                        out=h2[:], in0=h[:], scalar=1.0, in1=u[:],
                        op0=OP.bypass, op1=OP.mult)
                    nc.vector.tensor_scalar_add(h2[:], h2[:], 1.0)
                    lnT = sml.tile([P, 1], fp32, tag="lnT", bufs=2)
                    nc.vector.scalar_tensor_tensor(
                        out=lnT[:], in0=h2[:], scalar=1.0, in1=u[:],
                        op0=OP.bypass, op1=OP.mult)
                    # e = exp(l); v = (l > lnT) * e
                    e16 = e16p.tile([P, M], fp16, tag="e16")
                    nc.scalar.activation(e16[:], l16[:, bt, :], AF.Exp)
                    v16 = v16p.tile([P, MP], fp16, tag="v16")
                    if bt < 3:
                        nc.vector.memset(v16[:, M:MP], 0.0)
                    vs = sml.tile([P, 1], fp32, tag="vs", bufs=2)
                    nc.vector.scalar_tensor_tensor(
                        out=v16[:, :M], in0=l16[:, bt, :], scalar=lnT[:],
                        in1=e16[:], op0=OP.is_gt, op1=OP.mult, accum_out=vs[:])
                    nc.vector.reciprocal(invV[:, bt:bt + 1], vs[:])
                    vts[bt] = v16

                # ---- fused phase A + B1 ----
                with tc.tile_pool(name="xTp", bufs=1) as xTp, \
                     tc.tile_pool(name="mch", bufs=2) as mchp, \
                     tc.tile_pool(name="ain", bufs=2) as ainp, \
                     tc.tile_pool(name="ah16", bufs=2) as ah16p:
                    xT = xTp.tile([P, KT, B], fp16)

                    def x_prep(bt):
                        xin = ainp.tile([P, F], fp32, tag="ain", bufs=2)
                        nc.sync.dma_start(xin[:], xs[bt * P:(bt + 1) * P, :])
                        xh = ah16p.tile([P, F], fp16, tag="ah", bufs=2)
                        zsq = sml.tile([P, 1], fp32, tag="nsq", bufs=2)
                        nc.vector.scalar_tensor_tensor(
                            out=xh[:], in0=xin[:], scalar=1.0, in1=xin[:],
                            op0=OP.bypass, op1=OP.mult, accum_out=zsq[:])
                        s0 = sml.tile([P, 1], fp32, tag="s0", bufs=2)
                        nc.scalar.activation(s0[:], zsq[:], AF.Sqrt, bias=eps[:])
                        nc.vector.reciprocal(invz[:, bt:bt + 1], s0[:])
                        nc.scalar.activation(xh[:], xin[:], AF.Copy)
                        nc.scalar.dma_start_transpose(
                            xT[:, :, bt * P:(bt + 1) * P], xh[:])

                    def m_prep(mt, ct, j):
                        rows = min(P, M - mt * P)
                        min_ = ainp.tile([P, F], fp32, tag="ain", bufs=2)
                        if rows < P:
                            nc.vector.memset(min_[:], 0.0)
                            nc.sync.dma_start(min_[:rows, :],
                                              memory[mt * P:mt * P + rows, :])
                        else:
                            nc.sync.dma_start(min_[:],
                                              memory[mt * P:(mt + 1) * P, :])
                        mh = ah16p.tile([P, F], fp16, tag="ah", bufs=2)
                        msq = sml.tile([P, 1], fp32, tag="nsq", bufs=2)
                        nc.vector.scalar_tensor_tensor(
                            out=mh[:], in0=min_[:], scalar=1.0, in1=min_[:],
                            op0=OP.bypass, op1=OP.mult, accum_out=msq[:])
                        s0 = sml.tile([P, 1], fp32, tag="s0", bufs=2)
                        nc.scalar.activation(s0[:], msq[:], AF.Sqrt, bias=eps[:])
                        r0 = sml.tile([P, 1], fp32, tag="r0", bufs=2)
                        nc.vector.reciprocal(r0[:], s0[:])
                        s1 = sml.tile([P, 1], fp32, tag="s1", bufs=2)
                        nc.vector.scalar_tensor_tensor(
                            out=s1[:], in0=msq[:], scalar=r0[:], in1=s0[:],
                            op0=OP.mult, op1=OP.add)  # 2*norm (Newton)
                        im = sml.tile([P, 1], fp32, tag="im", bufs=2)
                        nc.vector.reciprocal(im[:], s1[:])
                        nc.vector.tensor_scalar_mul(im[:], im[:], 2.0)
                        nc.scalar.activation(mh[:], min_[:], AF.Copy, scale=im[:])
                        nc.scalar.dma_start_transpose(
                            ct[:, :, j * P:(j + 1) * P], mh[:])

                    def chunk_prep(c):
                        ct = mchp.tile([P, KT, 512], fp16, tag="mch", bufs=2)
                        for j in range(4):
                            m_prep(c * 4 + j, ct, j)
                        return ct

                    chunks = {}
                    x_prep(0)
                    x_prep(1)
                    chunks[0] = chunk_prep(0)
                    for bt in range(2, BT):
                        x_prep(bt)
                    chunks[1] = chunk_prep(1)

                    for c in range(NCH):
                        cols = min(512, M - c * 512)
                        ct = chunks.pop(c)
                        for bt in range(BT):
                            pt = ps.tile([P, 512], fp32, tag="pb")
                            for k in range(KT):
                                nc.tensor.matmul(
                                    pt[:, :cols],
                                    lhsT=xT[:, k, bt * P:(bt + 1) * P],
                                    rhs=ct[:, k, :cols],
                                    start=(k == 0), stop=(k == KT - 1))
                            # logit store (fp16) + exp row-sum
                            nc.vector.tensor_scalar_mul(
                                l16[:, bt, c * 512:c * 512 + cols],
                                pt[:, :cols], invz[:, bt:bt + 1])
                            dm = dmp.tile([P, 512], fp16, tag="dmp", bufs=2)
                            nc.scalar.activation(
                                dm[:, :cols], pt[:, :cols], AF.Exp,
                                scale=invz[:, bt:bt + 1],
                                accum_out=sacc[:, bt * NCH + c:bt * NCH + c + 1])
                            if c == NCH - 1:
                                thr_part(bt)
                        if c + 2 < NCH:
                            chunks[c + 2] = chunk_prep(c + 2)

                # ---- v transposes (alias freed xT/ain space) ----
                with tc.tile_pool(name="vTp", bufs=1) as vTp:
                    vT = []
                    for bt in range(BT):
                        vt = vTp.tile([P, MT, P], fp16, tag="vT", bufs=8,
                                      name=f"vT{bt}")
                        nc.sync.dma_start_transpose(vt[:], vts[bt][:])
                        vT.append(vt)

                    # ---- B2: out = (v/sum v) @ mem ----
                    with tc.tile_pool(name="natp", bufs=6) as natp, \
                         tc.tile_pool(name="evp", bufs=4) as evp:
                        for fc in range(FC):
                            pbs = []
                            for bt in range(BT):
                                pbs.append(ps.tile([P, 512], fp32, tag="pb",
                                                   name=f"pb2_{fc}_{bt}"))
                            for m in range(MT):
                                rows = min(P, M - m * P)
                                nt = natp.tile([P, 512], fp16, tag="nat", bufs=6)
                                if rows < P:
                                    nc.vector.memset(nt[:], 0.0)
                                    nc.gpsimd.dma_start(
                                        nt[:rows, :],
                                        memory[m * P:m * P + rows,
                                               fc * 512:(fc + 1) * 512])
                                else:
                                    nc.gpsimd.dma_start(
                                        nt[:],
                                        memory[m * P:(m + 1) * P,
                                               fc * 512:(fc + 1) * 512])
                                for bt in range(BT):
                                    nc.tensor.matmul(
                                        pbs[bt][:],
                                        lhsT=vT[bt][:, m, :],
                                        rhs=nt[:],
                                        start=(m == 0), stop=(m == MT - 1))
                            for bt in range(BT):
                                ev = evp.tile([P, 512], fp32, tag="ev", bufs=4)
                                nc.scalar.activation(
                                    ev[:], pbs[bt][:], AF.Copy,
                                    scale=invV[:, bt:bt + 1])
                                nc.sync.dma_start(
                                    out[bt * P:(bt + 1) * P,
                                        fc * 512:(fc + 1) * 512], ev[:])
    nc.compile()
    return nc


def _get_nc():
    if "nc" not in _CACHE:
        _CACHE["nc"] = build_nc()
    return _CACHE["nc"]


def kernel(x: np.ndarray, memory: np.ndarray) -> np.ndarray:
    from concourse.bass_utils import run_bass_kernel_spmd

    x = np.ascontiguousarray(x, dtype=np.float32)
    memory = np.ascontiguousarray(memory, dtype=np.float32)
    nc = _get_nc()
    in_maps = [
        {"xs": x[c * B:(c + 1) * B], "memory": memory} for c in range(N_CORES)
    ]
    res = run_bass_kernel_spmd(nc, in_maps, core_ids=list(range(N_CORES)))
    return np.concatenate([res.results[c]["out"] for c in range(N_CORES)], axis=0)


# revision 8
# speedup vs baseline: 1.0855x; 1.0855x over previous
"""Trainium2 Bass kernel for nn_MemoryUnit (cosine-sim memory read with sparse
softmax shrinkage), data-parallel over 8 NeuronCores.

Per core (batch shard of 1024 rows), single fused pipeline:

  prologue : load x tiles, cast fp16, DMA-xbar transpose -> xT resident
             [f,b]; row norms -> invz (folded into logit evict, not into x).
  fused A+B1 (m-chunk outer, 512 cols = 4 mem row-tiles per chunk):
             stream mem chunk, row norms (sqrt + Newton), normalized fp16,
             transpose -> mhatT chunk (rolling, bufs=2).  As soon as chunk c
             is up: 8 bt x 16 k matmuls accumulate logits[bt, chunk] in 8
             psum banks.  Evict: DVE copy psum*invz -> l16 (fp16 logit
             store; logits ~0 near the mask threshold so fp16 is exact
             enough), ScalarE exp(psum*invz) dumped with accum_out ->
             per-chunk row sums sacc.
  threshold: per bt (pipelined under the last chunk's matmuls):
             T = thr*S, lnT by 3-term Taylor (|T-1|<~0.01), e16=exp(l16),
             v16 = (l16 > lnT) * e16 with accum -> vsum, invV = 1/vsum.
             v transposed -> vT [m,b] resident (sync queue, overlaps B1 tail).
  B2       : out[b,f] = sum_m vT[m,b] * mem[m,f]; mem re-streamed from HBM
             with casting gpsimd DMA (fp32->fp16, no bounce buffer), fc-outer
             (4 waves of 512 f-cols) x 32 m k-tiles x 8 bt; evict scaled by
             invV (softmax S cancels algebraically).

Threshold identity: relu(w-t)*w/(|w-t|+1e-12) == w * 1{w>t} to ~1e-7 rel,
w = e/S, so mask is e > t*S <=> logit > ln(t*S); final L1 norm reduces to
division by sum(v).
"""
import sys

sys.path.insert(0, "/opt/trn_rl_repo")

import numpy as np

N_CORES = 8
B_FULL = 8192
B = B_FULL // N_CORES    # 1024 batch rows per core
M = 4000                 # memory rows
MP = 4096                # padded memory rows (transpose granularity)
F = 2048                 # features
P = 128

_CACHE = {}


def build_nc(B=B, M=M, MP=MP, F=F):
    import concourse.bacc as bacc
    import concourse.mybir as mybir
    import concourse.tile as tile

    fp32 = mybir.dt.float32
    fp16 = mybir.dt.float16
    AF = mybir.ActivationFunctionType
    OP = mybir.AluOpType

    KT = F // P              # 16 k-tiles (contraction over features)
    BT = B // P              # 8 batch tiles per core
    MT = MP // P             # 32 padded memory row-tiles
    NCH = MP // 512          # 8 m-chunks of 512 cols for B1
    FC = F // 512            # 4 f-chunks of 512 cols for B2
    thr = 1.0 / M

    nc = bacc.Bacc("TRN2", target_bir_lowering=False, debug=True)
    with tile.TileContext(nc) as tc:
        with tc.tile_pool(name="dram", bufs=1, space="DRAM") as dram:
            xs = dram.tile([B, F], fp32, kind="ExternalInput", uniquify=False, name="xs")
            memory = dram.tile([M, F], fp32, kind="ExternalInput", uniquify=False, name="memory")
            out = dram.tile([B, F], fp32, kind="ExternalOutput", uniquify=False, name="out")

            with tc.tile_pool(name="ps", bufs=8, space="PSUM") as ps, \
                 tc.tile_pool(name="stats", bufs=1) as stats, \
                 tc.tile_pool(name="sml", bufs=4) as sml, \
                 tc.tile_pool(name="dmp", bufs=2) as dmp, \
                 tc.tile_pool(name="l16p", bufs=1) as l16p, \
                 tc.tile_pool(name="e16p", bufs=2) as e16p, \
                 tc.tile_pool(name="v16p", bufs=3) as v16p:

                eps = stats.tile([P, 1], fp32)
                nc.gpsimd.memset(eps[:], 1e-30)
                invz = stats.tile([P, BT], fp32)
                invV = stats.tile([P, BT], fp32)
                sacc = stats.tile([P, BT * NCH], fp32)
                l16 = l16p.tile([P, BT, M], fp16)   # fp16 logit store

                vts = [None] * BT    # v16 tiles pending transpose
                ivs = [None] * BT

                def thr_part(bt):
                    # T = thr*S; u = T - 1
                    ds = sml.tile([P, NCH], fp32, tag="ds", bufs=2)
                    Tt = sml.tile([P, 1], fp32, tag="Tt", bufs=2)
                    nc.vector.tensor_scalar(
                        out=ds[:], in0=sacc[:, bt * NCH:(bt + 1) * NCH],
                        scalar1=thr, scalar2=0.0, op0=OP.mult,
                        op1=OP.add, accum_out=Tt[:])
                    u = sml.tile([P, 1], fp32, tag="u", bufs=2)
                    nc.vector.tensor_scalar_add(u[:], Tt[:], -1.0)
                    # lnT = ln(1+u) ~ u*(1 - u/2)  (|u| ~ 1e-2, err ~ u^3/3)
                    h = sml.tile([P, 1], fp32, tag="h", bufs=2)
                    nc.vector.tensor_scalar(
                        out=h[:], in0=u[:], scalar1=-0.5, scalar2=1.0,
                        op0=OP.mult, op1=OP.add)
                    lnT = sml.tile([P, 1], fp32, tag="lnT", bufs=2)
                    nc.vector.scalar_tensor_tensor(
                        out=lnT[:], in0=h[:], scalar=1.0, in1=u[:],
                        op0=OP.bypass, op1=OP.mult)
                    # e = exp(l); v = (l > lnT) * e
                    e16 = e16p.tile([P, M], fp16, tag="e16")
                    nc.scalar.activation(e16[:], l16[:, bt, :], AF.Exp)
                    v16 = v16p.tile([P, MP], fp16, tag="v16")
                    if bt < 3:
                        nc.vector.memset(v16[:, M:MP], 0.0)
                    vs = sml.tile([P, 1], fp32, tag="vs", bufs=2)
                    nc.vector.scalar_tensor_tensor(
                        out=v16[:, :M], in0=l16[:, bt, :], scalar=lnT[:],
                        in1=e16[:], op0=OP.is_gt, op1=OP.mult, accum_out=vs[:])
                    nc.vector.reciprocal(invV[:, bt:bt + 1], vs[:])
                    vts[bt] = v16

                # ---- fused phase A + B1 ----
                with tc.tile_pool(name="xTp", bufs=1) as xTp, \
                     tc.tile_pool(name="mch", bufs=2) as mchp, \
                     tc.tile_pool(name="ain", bufs=2) as ainp, \
                     tc.tile_pool(name="ah16", bufs=2) as ah16p:
                    xT = xTp.tile([P, KT, B], fp16)

                    def x_prep(bt):
                        xin = ainp.tile([P, F], fp32, tag="ain", bufs=2)
                        nc.sync.dma_start(xin[:], xs[bt * P:(bt + 1) * P, :])
                        xh = ah16p.tile([P, F], fp16, tag="ah", bufs=2)
                        zsq = sml.tile([P, 1], fp32, tag="nsq", bufs=2)
                        nc.vector.scalar_tensor_tensor(
                            out=xh[:], in0=xin[:], scalar=1.0, in1=xin[:],
                            op0=OP.bypass, op1=OP.mult, accum_out=zsq[:])
                        s0 = sml.tile([P, 1], fp32, tag="s0", bufs=2)
                        nc.scalar.activation(s0[:], zsq[:], AF.Sqrt, bias=eps[:])
                        nc.vector.reciprocal(invz[:, bt:bt + 1], s0[:])
                        nc.scalar.activation(xh[:], xin[:], AF.Copy)
                        nc.sync.dma_start_transpose(
                            xT[:, :, bt * P:(bt + 1) * P], xh[:])

                    def m_prep(mt, ct, j):
                        rows = min(P, M - mt * P)
                        min_ = ainp.tile([P, F], fp32, tag="ain", bufs=2)
                        if rows < P:
                            nc.vector.memset(min_[:], 0.0)
                            nc.sync.dma_start(min_[:rows, :],
                                              memory[mt * P:mt * P + rows, :])
                        else:
                            nc.sync.dma_start(min_[:],
                                              memory[mt * P:(mt + 1) * P, :])
                        mh = ah16p.tile([P, F], fp16, tag="ah", bufs=2)
                        msq = sml.tile([P, 1], fp32, tag="nsq", bufs=2)
                        nc.vector.scalar_tensor_tensor(
                            out=mh[:], in0=min_[:], scalar=1.0, in1=min_[:],
                            op0=OP.bypass, op1=OP.mult, accum_out=msq[:])
                        s0 = sml.tile([P, 1], fp32, tag="s0", bufs=2)
                        nc.scalar.activation(s0[:], msq[:], AF.Sqrt, bias=eps[:])
                        im = sml.tile([P, 1], fp32, tag="im", bufs=2)
                        nc.vector.reciprocal(im[:], s0[:])
                        nc.scalar.activation(mh[:], min_[:], AF.Copy, scale=im[:])
                        nc.sync.dma_start_transpose(
                            ct[:, :, j * P:(j + 1) * P], mh[:])

                    def chunk_prep(c):
                        ct = mchp.tile([P, KT, 512], fp16, tag="mch", bufs=2)
                        for j in range(4):
                            m_prep(c * 4 + j, ct, j)
                        return ct

                    chunks = {}
                    x_prep(0)
                    chunks[0] = chunk_prep(0)
                    x_prep(1)
                    x_prep(2)
                    chunks[1] = chunk_prep(1)
                    for bt in range(3, BT):
                        x_prep(bt)

                    for c in range(NCH):
                        cols = min(512, M - c * 512)
                        ct = chunks.pop(c)
                        for bt in range(BT):
                            pt = ps.tile([P, 512], fp32, tag="pb")
                            for k in range(KT):
                                nc.tensor.matmul(
                                    pt[:, :cols],
                                    lhsT=xT[:, k, bt * P:(bt + 1) * P],
                                    rhs=ct[:, k, :cols],
                                    start=(k == 0), stop=(k == KT - 1))
                            # logit store (fp16) + exp row-sum
                            nc.vector.tensor_scalar_mul(
                                l16[:, bt, c * 512:c * 512 + cols],
                                pt[:, :cols], invz[:, bt:bt + 1])
                            dm = dmp.tile([P, 512], fp16, tag="dmp", bufs=2)
                            nc.scalar.activation(
                                dm[:, :cols], pt[:, :cols], AF.Exp,
                                scale=invz[:, bt:bt + 1],
                                accum_out=sacc[:, bt * NCH + c:bt * NCH + c + 1])
                            if c == NCH - 1:
                                thr_part(bt)
                        if c + 2 < NCH:
                            chunks[c + 2] = chunk_prep(c + 2)

                # ---- v transposes (alias freed xT/ain space) ----
                with tc.tile_pool(name="vTp", bufs=1) as vTp:
                    vT = []
                    for bt in range(BT):
                        vt = vTp.tile([P, MT, P], fp16, tag="vT", bufs=8,
                                      name=f"vT{bt}")
                        nc.sync.dma_start_transpose(vt[:], vts[bt][:])
                        vT.append(vt)

                    # ---- B2: out = (v/sum v) @ mem ----
                    with tc.tile_pool(name="natp", bufs=6) as natp, \
                         tc.tile_pool(name="evp", bufs=4) as evp:
                        for fc in range(FC):
                            pbs = []
                            for bt in range(BT):
                                pbs.append(ps.tile([P, 512], fp32, tag="pb",
                                                   name=f"pb2_{fc}_{bt}"))
                            for m in range(MT):
                                rows = min(P, M - m * P)
                                nt = natp.tile([P, 512], fp16, tag="nat", bufs=6)
                                if rows < P:
                                    nc.vector.memset(nt[:], 0.0)
                                    nc.gpsimd.dma_start(
                                        nt[:rows, :],
                                        memory[m * P:m * P + rows,
                                               fc * 512:(fc + 1) * 512])
                                else:
                                    nc.gpsimd.dma_start(
                                        nt[:],
                                        memory[m * P:(m + 1) * P,
                                               fc * 512:(fc + 1) * 512])
                                for bt in range(BT):
                                    nc.tensor.matmul(
                                        pbs[bt][:],
                                        lhsT=vT[bt][:, m, :],
                                        rhs=nt[:],
                                        start=(m == 0), stop=(m == MT - 1))
                            for bt in range(BT):
                                ev = evp.tile([P, 512], fp32, tag="ev", bufs=4)
                                nc.scalar.activation(
                                    ev[:], pbs[bt][:], AF.Copy,
                                    scale=invV[:, bt:bt + 1])
                                nc.sync.dma_start(
                                    out[bt * P:(bt + 1) * P,
                                        fc * 512:(fc + 1) * 512], ev[:])
    nc.compile()
    return nc


def _get_nc():
    if "nc" not in _CACHE:
        _CACHE["nc"] = build_nc()
    return _CACHE["nc"]


def kernel(x: np.ndarray, memory: np.ndarray) -> np.ndarray:
    from concourse.bass_utils import run_bass_kernel_spmd

    x = np.ascontiguousarray(x, dtype=np.float32)
    memory = np.ascontiguousarray(memory, dtype=np.float32)
    nc = _get_nc()
    in_maps = [
        {"xs": x[c * B:(c + 1) * B], "memory": memory} for c in range(N_CORES)
    ]
    res = run_bass_kernel_spmd(nc, in_maps, core_ids=list(range(N_CORES)))
    return np.concatenate([res.results[c]["out"] for c in range(N_CORES)], axis=0)


# revision 10
# speedup vs baseline: 1.0982x; 1.0117x over previous
"""Trainium2 Bass kernel for nn_MemoryUnit (cosine-sim memory read with sparse
softmax shrinkage), data-parallel over 8 NeuronCores.

Per core (batch shard of 1024 rows), single fused pipeline:

  prologue : load x tiles, cast fp16, DMA-xbar transpose -> xT resident
             [f,b]; row norms -> invz (folded into logit evict, not into x).
  fused A+B1 (m-chunk outer, 512 cols = 4 mem row-tiles per chunk):
             stream mem chunk, row norms (sqrt + Newton), normalized fp16,
             transpose -> mhatT chunk (rolling, bufs=2).  As soon as chunk c
             is up: 8 bt x 16 k matmuls accumulate logits[bt, chunk] in 8
             psum banks.  Evict: DVE copy psum*invz -> l16 (fp16 logit
             store; logits ~0 near the mask threshold so fp16 is exact
             enough), ScalarE exp(psum*invz) dumped with accum_out ->
             per-chunk row sums sacc.
  threshold: per bt (pipelined under the last chunk's matmuls):
             T = thr*S, lnT by 3-term Taylor (|T-1|<~0.01), e16=exp(l16),
             v16 = (l16 > lnT) * e16 with accum -> vsum, invV = 1/vsum.
             v transposed -> vT [m,b] resident (sync queue, overlaps B1 tail).
  B2       : out[b,f] = sum_m vT[m,b] * mem[m,f]; mem re-streamed from HBM
             with casting gpsimd DMA (fp32->fp16, no bounce buffer), fc-outer
             (4 waves of 512 f-cols) x 32 m k-tiles x 8 bt; evict scaled by
             invV (softmax S cancels algebraically).

Threshold identity: relu(w-t)*w/(|w-t|+1e-12) == w * 1{w>t} to ~1e-7 rel,
w = e/S, so mask is e > t*S <=> logit > ln(t*S); final L1 norm reduces to
division by sum(v).
"""
import sys

sys.path.insert(0, "/opt/trn_rl_repo")

import numpy as np

N_CORES = 8
B_FULL = 8192
B = B_FULL // N_CORES    # 1024 batch rows per core
M = 4000                 # memory rows
MP = 4096                # padded memory rows (transpose granularity)
F = 2048                 # features
P = 128

_CACHE = {}


def build_nc(B=B, M=M, MP=MP, F=F):
    import concourse.bacc as bacc
    import concourse.mybir as mybir
    import concourse.tile as tile

    fp32 = mybir.dt.float32
    fp16 = mybir.dt.float16
    AF = mybir.ActivationFunctionType
    OP = mybir.AluOpType

    KT = F // P              # 16 k-tiles (contraction over features)
    BT = B // P              # 8 batch tiles per core
    MT = MP // P             # 32 padded memory row-tiles
    NCH = MP // 512          # 8 m-chunks of 512 cols for B1
    FC = F // 512            # 4 f-chunks of 512 cols for B2
    thr = 1.0 / M

    nc = bacc.Bacc("TRN2", target_bir_lowering=False, debug=True)
    with tile.TileContext(nc) as tc:
        with tc.tile_pool(name="dram", bufs=1, space="DRAM") as dram:
            xs = dram.tile([B, F], fp32, kind="ExternalInput", uniquify=False, name="xs")
            memory = dram.tile([M, F], fp32, kind="ExternalInput", uniquify=False, name="memory")
            out = dram.tile([B, F], fp32, kind="ExternalOutput", uniquify=False, name="out")

            with tc.tile_pool(name="ps", bufs=8, space="PSUM") as ps, \
                 tc.tile_pool(name="stats", bufs=1) as stats, \
                 tc.tile_pool(name="sml", bufs=4) as sml, \
                 tc.tile_pool(name="dmp", bufs=2) as dmp, \
                 tc.tile_pool(name="l16p", bufs=1) as l16p, \
                 tc.tile_pool(name="e16p", bufs=2) as e16p, \
                 tc.tile_pool(name="v16p", bufs=2) as v16p:

                eps = stats.tile([P, 1], fp32)
                nc.gpsimd.memset(eps[:], 1e-30)
                invz = stats.tile([P, BT], fp32)
                invV = stats.tile([P, BT], fp32)
                sacc = stats.tile([P, BT * NCH], fp32)
                l16 = l16p.tile([P, BT, M], fp16)   # fp16 logit store

                vts = [None] * BT    # v16 tiles pending transpose
                ivs = [None] * BT

                def thr_part(bt):
                    # T = thr*S; u = T - 1
                    ds = sml.tile([P, NCH], fp32, tag="ds", bufs=2)
                    Tt = sml.tile([P, 1], fp32, tag="Tt", bufs=2)
                    nc.vector.tensor_scalar(
                        out=ds[:], in0=sacc[:, bt * NCH:(bt + 1) * NCH],
                        scalar1=thr, scalar2=0.0, op0=OP.mult,
                        op1=OP.add, accum_out=Tt[:])
                    u = sml.tile([P, 1], fp32, tag="u", bufs=2)
                    nc.vector.tensor_scalar_add(u[:], Tt[:], -1.0)
                    # lnT = ln(1+u) ~ u*(1 - u/2)  (|u| ~ 1e-2, err ~ u^3/3)
                    h = sml.tile([P, 1], fp32, tag="h", bufs=2)
                    nc.vector.tensor_scalar(
                        out=h[:], in0=u[:], scalar1=-0.5, scalar2=1.0,
                        op0=OP.mult, op1=OP.add)
                    lnT = sml.tile([P, 1], fp32, tag="lnT", bufs=2)
                    nc.vector.scalar_tensor_tensor(
                        out=lnT[:], in0=h[:], scalar=1.0, in1=u[:],
                        op0=OP.bypass, op1=OP.mult)
                    # e = exp(l); v = (l > lnT) * e
                    e16 = e16p.tile([P, M], fp16, tag="e16")
                    nc.scalar.activation(e16[:], l16[:, bt, :], AF.Exp)
                    v16 = v16p.tile([P, MP], fp16, tag="v16")
                    if bt < 2:
                        nc.vector.memset(v16[:, M:MP], 0.0)
                    vs = sml.tile([P, 1], fp32, tag="vs", bufs=2)
                    nc.vector.scalar_tensor_tensor(
                        out=v16[:, :M], in0=l16[:, bt, :], scalar=lnT[:],
                        in1=e16[:], op0=OP.is_gt, op1=OP.mult, accum_out=vs[:])
                    nc.vector.reciprocal(invV[:, bt:bt + 1], vs[:])
                    vts[bt] = v16

                # ---- fused phase A + B1 ----
                with tc.tile_pool(name="xTp", bufs=1) as xTp, \
                     tc.tile_pool(name="mch", bufs=2) as mchp, \
                     tc.tile_pool(name="ain", bufs=2) as ainp, \
                     tc.tile_pool(name="ah16", bufs=2) as ah16p:
                    xT = xTp.tile([P, KT, B], fp16)
                    sqd = xTp.tile([P, F], fp16, name="sqd")

                    zsq8 = stats.tile([P, BT], fp32, name="zsq8")

                    def x_load(bt):
                        xin = ainp.tile([P, F], fp32, tag="ain", bufs=4,
                                        name=f"xin{bt}")
                        nc.sync.dma_start(xin[:], xs[bt * P:(bt + 1) * P, :])
                        return xin

                    def x_comp(bt, xin):
                        # squares dumped into xh, then overwritten by the cast
                        xh = ah16p.tile([P, F], fp16, tag="ah", bufs=2,
                                        name=f"xh{bt}")
                        nc.vector.scalar_tensor_tensor(
                            out=xh[:], in0=xin[:], scalar=1.0, in1=xin[:],
                            op0=OP.bypass, op1=OP.mult,
                            accum_out=zsq8[:, bt:bt + 1])
                        nc.scalar.activation(xh[:], xin[:], AF.Copy)
                        nc.sync.dma_start_transpose(
                            xT[:, :, bt * P:(bt + 1) * P], xh[:])

                    def chunk_load(c):
                        mins = []
                        for j in range(4):
                            mt = c * 4 + j
                            rows = min(P, M - mt * P)
                            min_ = ainp.tile([P, F], fp32, tag="ain", bufs=4,
                                             name=f"min{mt}")
                            if rows < P:
                                nc.vector.memset(min_[:], 0.0)
                                nc.sync.dma_start(
                                    min_[:rows, :],
                                    memory[mt * P:mt * P + rows, :])
                            else:
                                nc.sync.dma_start(
                                    min_[:], memory[mt * P:(mt + 1) * P, :])
                            mins.append(min_)
                        return mins

                    def chunk_comp(c, mins):
                        ct = mchp.tile([P, KT, 512], fp16, tag="mch", bufs=2,
                                       name=f"mch{c}")
                        nsq4 = sml.tile([P, 4], fp32, tag="nsq4", bufs=2,
                                        name=f"nsq4_{c}")
                        for j in range(4):
                            # write-only dump; only the accum row-sum is used
                            nc.vector.scalar_tensor_tensor(
                                out=sqd[:], in0=mins[j][:], scalar=1.0,
                                in1=mins[j][:], op0=OP.bypass, op1=OP.mult,
                                accum_out=nsq4[:, j:j + 1])
                        s4 = sml.tile([P, 4], fp32, tag="s4", bufs=2,
                                      name=f"s4_{c}")
                        nc.scalar.activation(s4[:], nsq4[:], AF.Sqrt,
                                             bias=eps[:])
                        im4 = sml.tile([P, 4], fp32, tag="im4", bufs=2,
                                       name=f"im4_{c}")
                        nc.vector.reciprocal(im4[:], s4[:])
                        mhs = []
                        for j in range(4):
                            mh = ah16p.tile([P, F], fp16, tag="ah", bufs=2,
                                            name=f"mh{c}_{j}")
                            nc.scalar.activation(mh[:], mins[j][:],
                                                 AF.Copy, scale=im4[:, j:j + 1])
                            mhs.append(mh)
                            nc.sync.dma_start_transpose(
                                ct[:, :, j * P:(j + 1) * P], mh[:])
                        return ct

                    def chunk_prep(c):
                        return chunk_comp(c, chunk_load(c))

                    # prologue: queue every load before any transpose can
                    # block the sync queue; x casts don't need norms (invz
                    # is folded into the logit evict), so buffers recycle.
                    chunks = {}
                    xl0 = x_load(0)
                    c0m = chunk_load(0)
                    xl1 = x_load(1)
                    x_comp(0, xl0)
                    chunks[0] = chunk_comp(0, c0m)
                    x_comp(1, xl1)
                    xl2 = x_load(2)
                    c1m = chunk_load(1)
                    x_comp(2, xl2)
                    xls = {}
                    for bt in range(3, BT):
                        xls[bt] = x_load(bt)
                    chunks[1] = chunk_comp(1, c1m)
                    for bt in range(3, BT):
                        x_comp(bt, xls[bt])
                    s8 = stats.tile([P, BT], fp32, name="s8")
                    nc.scalar.activation(s8[:], zsq8[:], AF.Sqrt, bias=eps[:])
                    nc.vector.reciprocal(invz[:], s8[:])

                    for c in range(NCH):
                        cols = min(512, M - c * 512)
                        ct = chunks.pop(c)
                        for bt in range(BT):
                            pt = ps.tile([P, 512], fp32, tag="pb")
                            for k in range(KT):
                                nc.tensor.matmul(
                                    pt[:, :cols],
                                    lhsT=xT[:, k, bt * P:(bt + 1) * P],
                                    rhs=ct[:, k, :cols],
                                    start=(k == 0), stop=(k == KT - 1))
                            # logit store (fp16) + exp row-sum
                            nc.vector.tensor_scalar_mul(
                                l16[:, bt, c * 512:c * 512 + cols],
                                pt[:, :cols], invz[:, bt:bt + 1])
                            dm = dmp.tile([P, 512], fp16, tag="dmp", bufs=2)
                            nc.scalar.activation(
                                dm[:, :cols], pt[:, :cols], AF.Exp,
                                scale=invz[:, bt:bt + 1],
                                accum_out=sacc[:, bt * NCH + c:bt * NCH + c + 1])
                            if c == NCH - 1:
                                thr_part(bt)
                        if c + 2 < NCH:
                            chunks[c + 2] = chunk_prep(c + 2)

                # ---- v transposes (alias freed xT/ain space) ----
                with tc.tile_pool(name="vTp", bufs=1) as vTp:
                    vT = []
                    for bt in range(BT):
                        vt = vTp.tile([P, MT, P], fp16, tag="vT", bufs=8,
                                      name=f"vT{bt}")
                        nc.sync.dma_start_transpose(vt[:], vts[bt][:])
                        vT.append(vt)

                    # ---- B2: out = (v/sum v) @ mem ----
                    with tc.tile_pool(name="natp", bufs=6) as natp, \
                         tc.tile_pool(name="evp", bufs=4) as evp:
                        for fc in range(FC):
                            pbs = []
                            for bt in range(BT):
                                pbs.append(ps.tile([P, 512], fp32, tag="pb",
                                                   name=f"pb2_{fc}_{bt}"))
                            for m in range(MT):
                                rows = min(P, M - m * P)
                                nt = natp.tile([P, 512], fp16, tag="nat", bufs=6)
                                if rows < P:
                                    nc.vector.memset(nt[:], 0.0)
                                    nc.gpsimd.dma_start(
                                        nt[:rows, :],
                                        memory[m * P:m * P + rows,
                                               fc * 512:(fc + 1) * 512])
                                else:
                                    nc.gpsimd.dma_start(
                                        nt[:],
                                        memory[m * P:(m + 1) * P,
                                               fc * 512:(fc + 1) * 512])
                                for bt in range(BT):
                                    nc.tensor.matmul(
                                        pbs[bt][:],
                                        lhsT=vT[bt][:, m, :],
                                        rhs=nt[:],
                                        start=(m == 0), stop=(m == MT - 1))
                            for bt in range(BT):
                                ev = evp.tile([P, 512], fp32, tag="ev", bufs=4)
                                nc.scalar.activation(
                                    ev[:], pbs[bt][:], AF.Copy,
                                    scale=invV[:, bt:bt + 1])
                                nc.sync.dma_start(
                                    out[bt * P:(bt + 1) * P,
                                        fc * 512:(fc + 1) * 512], ev[:])
    nc.compile()
    return nc


def _get_nc():
    if "nc" not in _CACHE:
        _CACHE["nc"] = build_nc()
    return _CACHE["nc"]


def kernel(x: np.ndarray, memory: np.ndarray) -> np.ndarray:
    from concourse.bass_utils import run_bass_kernel_spmd

    x = np.ascontiguousarray(x, dtype=np.float32)
    memory = np.ascontiguousarray(memory, dtype=np.float32)
    nc = _get_nc()
    in_maps = [
        {"xs": x[c * B:(c + 1) * B], "memory": memory} for c in range(N_CORES)
    ]
    res = run_bass_kernel_spmd(nc, in_maps, core_ids=list(range(N_CORES)))
    return np.concatenate([res.results[c]["out"] for c in range(N_CORES)], axis=0)


# revision 13
# speedup vs baseline: 1.1337x; 1.0323x over previous
"""Trainium2 Bass kernel for nn_MemoryUnit (cosine-sim memory read with sparse
softmax shrinkage), data-parallel over 8 NeuronCores.

Per core (batch shard of 1024 rows), single fused pipeline:

  prologue : load x tiles, cast fp16, DMA-xbar transpose -> xT resident
             [f,b]; row norms -> invz (folded into logit evict, not into x).
  fused A+B1 (m-chunk outer, 512 cols = 4 mem row-tiles per chunk):
             stream mem chunk, row norms (sqrt + Newton), normalized fp16,
             transpose -> mhatT chunk (rolling, bufs=2).  As soon as chunk c
             is up: 8 bt x 16 k matmuls accumulate logits[bt, chunk] in 8
             psum banks.  Evict: DVE copy psum*invz -> l16 (fp16 logit
             store; logits ~0 near the mask threshold so fp16 is exact
             enough), ScalarE exp(psum*invz) dumped with accum_out ->
             per-chunk row sums sacc.
  threshold: per bt (pipelined under the last chunk's matmuls):
             T = thr*S, lnT by 3-term Taylor (|T-1|<~0.01), e16=exp(l16),
             v16 = (l16 > lnT) * e16 with accum -> vsum, invV = 1/vsum.
             v transposed -> vT [m,b] resident (sync queue, overlaps B1 tail).
  B2       : out[b,f] = sum_m vT[m,b] * mem[m,f]; mem re-streamed from HBM
             with casting gpsimd DMA (fp32->fp16, no bounce buffer), fc-outer
             (4 waves of 512 f-cols) x 32 m k-tiles x 8 bt; evict scaled by
             invV (softmax S cancels algebraically).

Threshold identity: relu(w-t)*w/(|w-t|+1e-12) == w * 1{w>t} to ~1e-7 rel,
w = e/S, so mask is e > t*S <=> logit > ln(t*S); final L1 norm reduces to
division by sum(v).
"""
import sys

sys.path.insert(0, "/opt/trn_rl_repo")

import numpy as np

N_CORES = 8
B_FULL = 8192
B = B_FULL // N_CORES    # 1024 batch rows per core
M = 4000                 # memory rows
MP = 4096                # padded memory rows (transpose granularity)
F = 2048                 # features
P = 128

_CACHE = {}


def build_nc(B=B, M=M, MP=MP, F=F):
    import concourse.bacc as bacc
    import concourse.mybir as mybir
    import concourse.tile as tile

    fp32 = mybir.dt.float32
    fp16 = mybir.dt.float16
    AF = mybir.ActivationFunctionType
    OP = mybir.AluOpType

    KT = F // P              # 16 k-tiles (contraction over features)
    BT = B // P              # 8 batch tiles per core
    MT = MP // P             # 32 padded memory row-tiles
    NCH = MP // 512          # 8 m-chunks of 512 cols for B1
    FC = F // 512            # 4 f-chunks of 512 cols for B2
    thr = 1.0 / M

    nc = bacc.Bacc("TRN2", target_bir_lowering=False, debug=True)
    with tile.TileContext(nc) as tc:
        with tc.tile_pool(name="dram", bufs=1, space="DRAM") as dram:
            xs = dram.tile([B, F], fp32, kind="ExternalInput", uniquify=False, name="xs")
            memory = dram.tile([M, F], fp32, kind="ExternalInput", uniquify=False, name="memory")
            out = dram.tile([B, F], fp32, kind="ExternalOutput", uniquify=False, name="out")

            with tc.tile_pool(name="ps", bufs=8, space="PSUM") as ps, \
                 tc.tile_pool(name="stats", bufs=1) as stats, \
                 tc.tile_pool(name="sml", bufs=4) as sml, \
                 tc.tile_pool(name="dmp", bufs=2) as dmp, \
                 tc.tile_pool(name="l16p", bufs=1) as l16p, \
                 tc.tile_pool(name="e16p", bufs=2) as e16p, \
                 tc.tile_pool(name="v16p", bufs=2) as v16p:

                eps = stats.tile([P, 1], fp32)
                nc.gpsimd.memset(eps[:], 1e-30)
                invz = stats.tile([P, BT], fp32)
                invV = stats.tile([P, BT], fp32)
                sacc = stats.tile([P, BT * NCH], fp32)
                l16 = l16p.tile([P, BT, M], fp16)   # fp16 logit store

                vts = [None] * BT    # v16 tiles pending transpose
                ivs = [None] * BT

                def thr_part(bt):
                    # T = thr*S; u = T - 1
                    ds = sml.tile([P, NCH], fp32, tag="ds", bufs=2)
                    Tt = sml.tile([P, 1], fp32, tag="Tt", bufs=2)
                    nc.vector.tensor_scalar(
                        out=ds[:], in0=sacc[:, bt * NCH:(bt + 1) * NCH],
                        scalar1=thr, scalar2=0.0, op0=OP.mult,
                        op1=OP.add, accum_out=Tt[:])
                    u = sml.tile([P, 1], fp32, tag="u", bufs=2)
                    nc.vector.tensor_scalar_add(u[:], Tt[:], -1.0)
                    # lnT = ln(1+u) ~ u*(1 - u/2)  (|u| ~ 1e-2, err ~ u^3/3)
                    h = sml.tile([P, 1], fp32, tag="h", bufs=2)
                    nc.vector.tensor_scalar(
                        out=h[:], in0=u[:], scalar1=-0.5, scalar2=1.0,
                        op0=OP.mult, op1=OP.add)
                    lnT = sml.tile([P, 1], fp32, tag="lnT", bufs=2)
                    nc.vector.scalar_tensor_tensor(
                        out=lnT[:], in0=h[:], scalar=1.0, in1=u[:],
                        op0=OP.bypass, op1=OP.mult)
                    # e = exp(l); v = (l > lnT) * e
                    e16 = e16p.tile([P, M], fp16, tag="e16")
                    nc.scalar.activation(e16[:], l16[:, bt, :], AF.Exp)
                    v16 = v16p.tile([P, MP], fp16, tag="v16")
                    if bt < 2:
                        nc.vector.memset(v16[:, M:MP], 0.0)
                    vs = sml.tile([P, 1], fp32, tag="vs", bufs=2)
                    nc.vector.scalar_tensor_tensor(
                        out=v16[:, :M], in0=l16[:, bt, :], scalar=lnT[:],
                        in1=e16[:], op0=OP.is_gt, op1=OP.mult, accum_out=vs[:])
                    nc.vector.reciprocal(invV[:, bt:bt + 1], vs[:])
                    vts[bt] = v16

                # ---- fused phase A + B1 ----
                with tc.tile_pool(name="xTp", bufs=1) as xTp, \
                     tc.tile_pool(name="mch", bufs=2) as mchp, \
                     tc.tile_pool(name="ain", bufs=4) as ainp, \
                     tc.tile_pool(name="ah16", bufs=2) as ah16p:
                    xT = xTp.tile([P, KT, B], fp16)
                    sqd = xTp.tile([P, F], fp16, name="sqd")

                    zsq8 = stats.tile([P, BT], fp32, name="zsq8")

                    def x_load(bt):
                        xin = ainp.tile([P, F], fp32, tag="ain", bufs=4,
                                        name=f"xin{bt}")
                        nc.sync.dma_start(xin[:], xs[bt * P:(bt + 1) * P, :])
                        return xin

                    def x_comp(bt, xin):
                        # squares on gpsimd (idle in B1); cast on scalar
                        nc.vector.scalar_tensor_tensor(
                            out=sqd[:], in0=xin[:], scalar=1.0, in1=xin[:],
                            op0=OP.bypass, op1=OP.mult,
                            accum_out=zsq8[:, bt:bt + 1])
                        xh = ah16p.tile([P, F], fp16, tag="ah", bufs=2,
                                        name=f"xh{bt}")
                        nc.scalar.activation(xh[:], xin[:], AF.Copy)
                        nc.sync.dma_start_transpose(
                            xT[:, :, bt * P:(bt + 1) * P], xh[:])

                    def chunk_load(c):
                        mins = []
                        for j in range(4):
                            mt = c * 4 + j
                            rows = min(P, M - mt * P)
                            min_ = ainp.tile([P, F], fp32, tag="ain", bufs=4,
                                             name=f"min{mt}")
                            if rows < P:
                                nc.vector.memset(min_[:], 0.0)
                                nc.sync.dma_start(
                                    min_[:rows, :],
                                    memory[mt * P:mt * P + rows, :])
                            else:
                                nc.sync.dma_start(
                                    min_[:], memory[mt * P:(mt + 1) * P, :])
                            mins.append(min_)
                        return mins

                    def chunk_comp(c, mins):
                        ct = mchp.tile([P, KT, 512], fp16, tag="mch", bufs=2,
                                       name=f"mch{c}")
                        nsq4 = sml.tile([P, 4], fp32, tag="nsq4", bufs=2,
                                        name=f"nsq4_{c}")
                        for j in range(4):
                            # write-only dump; only the accum row-sum is used
                            nc.vector.scalar_tensor_tensor(
                                out=sqd[:], in0=mins[j][:], scalar=1.0,
                                in1=mins[j][:], op0=OP.bypass, op1=OP.mult,
                                accum_out=nsq4[:, j:j + 1])
                        s4 = sml.tile([P, 4], fp32, tag="s4", bufs=2,
                                      name=f"s4_{c}")
                        nc.scalar.activation(s4[:], nsq4[:], AF.Sqrt,
                                             bias=eps[:])
                        im4 = sml.tile([P, 4], fp32, tag="im4", bufs=2,
                                       name=f"im4_{c}")
                        nc.vector.reciprocal(im4[:], s4[:])
                        for j in range(4):
                            mh = ah16p.tile([P, F], fp16, tag="ah", bufs=2,
                                            name=f"mh{c}_{j}")
                            nc.scalar.activation(mh[:], mins[j][:], AF.Copy,
                                                 scale=im4[:, j:j + 1])
                            eng = nc.sync if j < 2 else nc.scalar
                            eng.dma_start_transpose(
                                ct[:, :, j * P:(j + 1) * P], mh[:])
                        return ct

                    def chunk_prep(c):
                        return chunk_comp(c, chunk_load(c))

                    # prologue: all loads queued before any transpose
                    chunks = {}
                    xl = {0: x_load(0)}
                    c0m = chunk_load(0)
                    xl[1] = x_load(1)
                    xl[2] = x_load(2)
                    c1m = chunk_load(1)
                    for bt in range(3, BT):
                        xl[bt] = x_load(bt)
                    x_comp(0, xl[0])
                    chunks[0] = chunk_comp(0, c0m)
                    x_comp(1, xl[1])
                    x_comp(2, xl[2])
                    chunks[1] = chunk_comp(1, c1m)
                    for bt in range(3, BT):
                        x_comp(bt, xl[bt])
                    s8 = stats.tile([P, BT], fp32, name="s8")
                    nc.scalar.activation(s8[:], zsq8[:], AF.Sqrt, bias=eps[:])
                    nc.vector.reciprocal(invz[:], s8[:])

                    for c in range(NCH):
                        cols = min(512, M - c * 512)
                        ct = chunks.pop(c)
                        for bt in range(BT):
                            pt = ps.tile([P, 512], fp32, tag="pb")
                            for k in range(KT):
                                nc.tensor.matmul(
                                    pt[:, :cols],
                                    lhsT=xT[:, k, bt * P:(bt + 1) * P],
                                    rhs=ct[:, k, :cols],
                                    start=(k == 0), stop=(k == KT - 1))
                            # logit store (fp16) + exp row-sum
                            nc.vector.tensor_scalar_mul(
                                l16[:, bt, c * 512:c * 512 + cols],
                                pt[:, :cols], invz[:, bt:bt + 1])
                            dm = dmp.tile([P, 512], fp16, tag="dmp", bufs=2)
                            nc.scalar.activation(
                                dm[:, :cols], pt[:, :cols], AF.Exp,
                                scale=invz[:, bt:bt + 1],
                                accum_out=sacc[:, bt * NCH + c:bt * NCH + c + 1])
                            if c == NCH - 1:
                                thr_part(bt)
                        if c + 2 < NCH:
                            chunks[c + 2] = chunk_prep(c + 2)

                # ---- v transposes (alias freed xT/ain space) ----
                with tc.tile_pool(name="vTp", bufs=1) as vTp:
                    vT = []
                    for bt in range(BT):
                        vt = vTp.tile([P, MT, P], fp16, tag="vT", bufs=8,
                                      name=f"vT{bt}")
                        nc.sync.dma_start_transpose(vt[:], vts[bt][:])
                        vT.append(vt)

                    # ---- B2: out = (v/sum v) @ mem ----
                    with tc.tile_pool(name="natp", bufs=6) as natp, \
                         tc.tile_pool(name="evp", bufs=4) as evp:
                        for fc in range(FC):
                            pbs = []
                            for bt in range(BT):
                                pbs.append(ps.tile([P, 512], fp32, tag="pb",
                                                   name=f"pb2_{fc}_{bt}"))
                            for m in range(MT):
                                rows = min(P, M - m * P)
                                nt = natp.tile([P, 512], fp16, tag="nat", bufs=6)
                                if rows < P:
                                    nc.vector.memset(nt[:], 0.0)
                                    nc.gpsimd.dma_start(
                                        nt[:rows, :],
                                        memory[m * P:m * P + rows,
                                               fc * 512:(fc + 1) * 512])
                                else:
                                    nc.gpsimd.dma_start(
                                        nt[:],
                                        memory[m * P:(m + 1) * P,
                                               fc * 512:(fc + 1) * 512])
                                for bt in range(BT):
                                    nc.tensor.matmul(
                                        pbs[bt][:],
                                        lhsT=vT[bt][:, m, :],
                                        rhs=nt[:],
                                        start=(m == 0), stop=(m == MT - 1))
                            for bt in range(BT):
                                ev = evp.tile([P, 512], fp32, tag="ev", bufs=4)
                                nc.scalar.activation(
                                    ev[:], pbs[bt][:], AF.Copy,
                                    scale=invV[:, bt:bt + 1])
                                nc.sync.dma_start(
                                    out[bt * P:(bt + 1) * P,
                                        fc * 512:(fc + 1) * 512], ev[:])
    nc.compile()
    return nc


def _get_nc():
    if "nc" not in _CACHE:
        _CACHE["nc"] = build_nc()
    return _CACHE["nc"]


def kernel(x: np.ndarray, memory: np.ndarray) -> np.ndarray:
    from concourse.bass_utils import run_bass_kernel_spmd

    x = np.ascontiguousarray(x, dtype=np.float32)
    memory = np.ascontiguousarray(memory, dtype=np.float32)
    nc = _get_nc()
    in_maps = [
        {"xs": x[c * B:(c + 1) * B], "memory": memory} for c in range(N_CORES)
    ]
    res = run_bass_kernel_spmd(nc, in_maps, core_ids=list(range(N_CORES)))
    return np.concatenate([res.results[c]["out"] for c in range(N_CORES)], axis=0)
